# revision 16
# baseline (speedup 1.0000x reference)
"""Trainium2 Bass kernel for nn_Attention_84585085927925 — W_eff variant.

Reference (per batch element b, all fp32):
    qkv = x @ w_qkv.T ; q,k,v heads of 64 ; attn = sqrt(64) * q @ k.T (NO
    softmax) ; out = attn @ v ; out = out @ w_fc.T + b_fc

With no softmax the whole module is linear in x on the left:
    out = x @ W_eff + b_fc,
    W_eff = sum_h (s*w_q_h)^T (w_k_h C w_v_h^T) (w_fc^T)_h,   C = x^T x
so q/k/v are never materialized. Per-core pipeline (one batch element per
NeuronCore, 8 cores, no collectives), everything bf16 into fp32 PSUM:
    C    = x^T x           upper blocks only; lower via PE transpose
    T1   = C @ wv^T        [768,768]
    GT_t = T1_t^T @ wkT_t  per head pair; keep diagonal 64x64 blocks
    M_t  = G2_t @ F_t      F = wfc^T rows; block-diag stationary
    Weff = (s*wq)^T @ M    [768,768]
    outT = Weff^T x^T + b  [768,1024] -> bf16 out
Host does layout transposes + bf16 casts. ~122K PE cycles/core.

Scheduling notes:
 - xN arrives in 4 chunks ([1,1,2,4] n-tiles); C runs nt-outer in 3
   row-pass groups so the PE consumes chunks in arrival order.
 - The PE p-state ramps on a single accumulating warmup matmul group
   (reads a memset dummy; no DMA dependency) while xN chunk 0 is in
   flight; PE gaps reset the clock ramp, so stages are kept dense.
 - C lower blocks are produced by PE transposes (identity built on
   device); bias/wq/xT loads are issued mid-stream in consumption order.
 - The last out row-block is split 512/384/128 with the final add+DMA
   pair on the Activation queue so the dependent tail is short.
"""

import numpy as np
import ml_dtypes

import concourse.bass as bass  # noqa: F401  (registers engine namespaces)
import concourse.mybir as mybir
import concourse.tile as tile
from concourse import bacc, bass_utils
from concourse.masks import make_identity

F32 = mybir.dt.float32
BF16 = mybir.dt.bfloat16
NPBF16 = ml_dtypes.bfloat16

B, N, D, H = 8, 1024, 768, 12
HD = D // H            # 64
SCALE = float(np.sqrt(HD))
DT = D // 128          # 6  d-tiles
NT = N // 128          # 8  n(token)-tiles
NPAIR = H // 2         # 6 head pairs

XN_CHUNKS = [(0, 1), (1, 1), (2, 2), (4, 4)]   # (start nt, n nt)
C_PASSES = [(0, 1, 2), (3, 4, 5)]              # row groups per nt sweep
N_WARMUP = 8


def _chunks(lo, hi, step=512):
    out = []
    while lo < hi:
        out.append((lo, min(step, hi - lo)))
        lo += step
    return out


def _build_program():
    nc = bacc.Bacc(
        trn_type="TRN2", target_bir_lowering=False, debug=False, num_devices=B
    )
    xN_d = nc.dram_tensor("xN", [N, D], BF16, kind="ExternalInput").ap()
    xT_d = nc.dram_tensor("xT", [D, N], BF16, kind="ExternalInput").ap()
    wq_d = nc.dram_tensor("wq", [D, D], BF16, kind="ExternalInput").ap()
    wvT_d = nc.dram_tensor("wvT", [D, D], BF16, kind="ExternalInput").ap()
    wkT_d = nc.dram_tensor("wkT", [D, D], BF16, kind="ExternalInput").ap()
    wfcT_d = nc.dram_tensor("wfcT", [D, D], BF16, kind="ExternalInput").ap()
    bfc_d = nc.dram_tensor("bfc", [D], F32, kind="ExternalInput").ap()
    outT_d = nc.dram_tensor("outT", [D, N], BF16, kind="ExternalOutput").ap()

    xN_r = xN_d.rearrange("(o p) d -> p o d", p=128)
    xT_r = xT_d.rearrange("(o p) n -> p o n", p=128)
    wq_r = wq_d.rearrange("(o p) d -> p o d", p=128)
    wvT_r = wvT_d.rearrange("(o p) j -> p o j", p=128)
    wkT_r = wkT_d.rearrange("(o p) j -> p o j", p=128)
    wfcT_r = wfcT_d.rearrange("(o p) e -> p o e", p=128)
    outT_r = outT_d.rearrange("(o p) n -> p o n", p=128)

    with tile.TileContext(nc) as tc:
        with tc.tile_pool(name="big", bufs=1) as big, \
             tc.tile_pool(name="outsp", bufs=3) as outsp, \
             tc.tile_pool(name="ps", bufs=7, space="PSUM") as ps:

            xN_sb = big.tile([128, NT, D], BF16, name="xN_sb")
            xT_sb = big.tile([128, DT, N], BF16, name="xT_sb")
            c_sb = big.tile([128, DT, D], BF16, name="c_sb")
            t1_sb = big.tile([128, DT, D], BF16, name="t1_sb")
            g2T_sb = big.tile([128, NPAIR, 128], BF16, name="g2T_sb")
            m_sb = big.tile([128, NPAIR, D], BF16, name="m_sb")
            weff_sb = big.tile([128, DT, D], BF16, name="weff_sb")
            wq_sb = big.tile([128, DT, D], BF16, name="wq_sb")
            wvT_sb = big.tile([128, DT, D], BF16, name="wvT_sb")
            wkT_sb = big.tile([128, DT, D], BF16, name="wkT_sb")
            wfcT_sb = big.tile([128, NPAIR, D], BF16, name="wfcT_sb")
            bias_sb = big.tile([128, DT], F32, name="bias_sb")
            dummy_sb = big.tile([128, 384], BF16, name="dummy_sb")
            ident_sb = big.tile([128, 128], BF16, name="ident_sb")

            # ---- early input DMAs: chunk 0 split across the SP and Act
            # queues so both DGE spin-ups run in parallel and the first
            # C matmul's data lands sooner; rest on SP in arrival order
            nc.sync.dma_start(xN_sb[0:64, 0:1, :], xN_r[0:64, 0:1, :])
            nc.scalar.dma_start(xN_sb[64:128, 0:1, :], xN_r[64:128, 0:1, :])
            for nt0, nn in XN_CHUNKS[1:]:
                nc.sync.dma_start(xN_sb[:, nt0:nt0 + nn, :],
                                  xN_r[:, nt0:nt0 + nn, :])
            nc.sync.dma_start(wvT_sb[:], wvT_r)
            nc.sync.dma_start(wkT_sb[:], wkT_r)
            nc.sync.dma_start(wfcT_sb[:], wfcT_r)

            # ---- PE p-state warmup: one accumulating matmul group ----
            nc.gpsimd.memset(dummy_sb[:], 0.0)
            make_identity(nc, ident_sb[:])
            nc.vector.memset(g2T_sb[:], 0.0)
            pw = ps.tile([128, 384], F32, tag="bank", name="pw")
            for k in range(N_WARMUP):
                nc.tensor.matmul(
                    pw[:], dummy_sb[:, 0:128], dummy_sb[:],
                    start=(k == 0), stop=(k == N_WARMUP - 1),
                )

            # ---- C = x^T x, upper blocks; nt-outer passes over row groups -
            # c_sb[p, i, c] = C[i*128+p, c]
            row_cols = {i: _chunks(i * 128, D) for i in range(DT)}
            row_psum = {}
            for rows in C_PASSES:
                for i in rows:
                    row_psum[i] = [
                        ps.tile([128, w], F32, tag="bank", name=f"pt_c{i}")
                        for (_, w) in row_cols[i]
                    ]
                for nt in range(NT):
                    for i in rows:
                        lhs = xN_sb[:, nt, i * 128:(i + 1) * 128]
                        for (off, w), pt in zip(row_cols[i], row_psum[i]):
                            nc.tensor.matmul(
                                pt[:], lhs, xN_sb[:, nt, off:off + w],
                                start=(nt == 0), stop=(nt == NT - 1),
                            )
                for i in rows:
                    for (off, w), pt in zip(row_cols[i], row_psum[i]):
                        nc.vector.tensor_copy(c_sb[:, i, off:off + w], pt[:])

            # ---- late input DMAs, in consumption order ----
            nc.sync.dma_start(bias_sb[:], bfc_d.rearrange("(o p) -> p o", p=128))
            nc.sync.dma_start(wq_sb[:], wq_r)

            # lower block (j, i) = PE transpose of upper (i, j); row-major so
            # T1 panel d1 (which consumes row-d1 transposes) is unblocked in
            # order; last transposes only need the last C pass's copies
            # all row-j lowers transpose into one wide bf16 psum tile
            # (bf16 pass-through: 640 cols still fit one bank), then a
            # single batched copy per j keeps the copy chain off the PE
            for j in range(1, DT):
                tpt = ps.tile([128, j * 128], BF16, tag="bank", name="tp_c")
                for i in range(j):
                    nc.tensor.transpose(
                        tpt[:, i * 128:(i + 1) * 128],
                        c_sb[:, i, j * 128:(j + 1) * 128], ident_sb[:])
                dst = c_sb[:, j, 0:j * 128]
                if j % 2 == 0:
                    nc.vector.tensor_copy(dst, tpt[:])
                else:
                    nc.scalar.copy(dst, tpt[:])

            # ---- T1 = C @ wv^T : T1[d1, j'] = sum_d2 C[d2, d1] wvT[d2, j'] --
            for d1 in range(DT):
                cols = _chunks(0, D)
                pts = [ps.tile([128, w], F32, tag="bank", name="pt_t1")
                       for (_, w) in cols]
                for d2 in range(DT):
                    lhs = c_sb[:, d2, d1 * 128:(d1 + 1) * 128]
                    for (off, w), pt in zip(cols, pts):
                        nc.tensor.matmul(
                            pt[:], lhs, wvT_sb[:, d2, off:off + w],
                            start=(d2 == 0), stop=(d2 == DT - 1),
                        )
                if d1 < DT - 1:
                    nc.scalar.copy(t1_sb[:, d1, 0:512], pts[0][:])
                    nc.vector.tensor_copy(t1_sb[:, d1, 512:768], pts[1][:])
                else:
                    # last panel: GT pair t needs cols [t*128,(t+1)*128) of
                    # every panel — copy pair-0's slice first, split engines
                    nc.scalar.copy(t1_sb[:, d1, 0:128], pts[0][:, 0:128])
                    nc.vector.tensor_copy(t1_sb[:, d1, 128:512],
                                          pts[0][:, 128:512])
                    nc.scalar.copy(t1_sb[:, d1, 512:768], pts[1][:])
                if d1 == 2:
                    # gate xT's big load to start mid-T1 so its transfer
                    # runs in the DMA-idle GT/M/Weff window (outT needs it
                    # only ~20us later), not against T1's SBUF traffic
                    nc.vector.memset(xT_sb[0:1, 0:1, 0:1], 0.0)
                    nc.sync.dma_start(xT_sb[:], xT_r)

            # ---- GT per pair (gt = T1_pair^T @ wkT_pair), M lag-1 behind
            def emit_gt(t):
                gt = ps.tile([128, 128], F32, tag="bank", name="gt")
                pc = slice(t * 128, (t + 1) * 128)
                for dt in range(DT):
                    nc.tensor.matmul(
                        gt[:], t1_sb[:, dt, pc], wkT_sb[:, dt, pc],
                        start=(dt == 0), stop=(dt == DT - 1),
                    )
                if t % 2 == 0:
                    nc.vector.tensor_copy(g2T_sb[0:64, t, 0:64],
                                          gt[0:64, 0:64])
                    nc.vector.tensor_copy(g2T_sb[64:128, t, 64:128],
                                          gt[64:128, 64:128])
                else:
                    nc.scalar.copy(g2T_sb[0:64, t, 0:64], gt[0:64, 0:64])
                    nc.scalar.copy(g2T_sb[64:128, t, 64:128],
                                   gt[64:128, 64:128])

            def emit_m(t):
                cols = _chunks(0, D)
                for k, (off, w) in enumerate(cols):
                    pm = ps.tile([128, w], F32, tag="bank", name="pt_m")
                    nc.tensor.matmul(
                        pm[:], g2T_sb[:, t, :], wfcT_sb[:, t, off:off + w],
                        start=True, stop=True,
                    )
                    if k == 0:
                        nc.scalar.copy(m_sb[:, t, off:off + w], pm[:])
                    else:
                        nc.vector.tensor_copy(m_sb[:, t, off:off + w], pm[:])

            for t in range(NPAIR):
                emit_gt(t)
            for t in range(NPAIR):
                emit_m(t)

            # ---- Weff[d, e] = sum_j wq_s[j, d] M[j, e] ----
            for dt in range(DT):
                cols = _chunks(0, D)
                pts = [ps.tile([128, w], F32, tag="bank", name="pt_w")
                       for (_, w) in cols]
                for jt in range(DT):
                    lhs = wq_sb[:, jt, dt * 128:(dt + 1) * 128]
                    for (off, w), pt in zip(cols, pts):
                        nc.tensor.matmul(
                            pt[:], lhs, m_sb[:, jt, off:off + w],
                            start=(jt == 0), stop=(jt == DT - 1),
                        )
                for k, ((off, w), pt) in enumerate(zip(cols, pts)):
                    if k == 0:
                        nc.scalar.copy(weff_sb[:, dt, off:off + w], pt[:])
                    else:
                        nc.vector.tensor_copy(weff_sb[:, dt, off:off + w],
                                              pt[:])

            # ---- outT[e, n] = sum_d Weff[d, e] xT[d, n] + b[e] ----
            for et in range(DT):
                last = (et == DT - 1)
                cols = [(0, 512), (512, 448), (960, 64)] if last \
                    else [(0, 512), (512, 512)]
                pts = [ps.tile([128, w], F32, tag="bank", name="pt_o")
                       for (_, w) in cols]
                ot = outsp.tile([128, N], BF16, tag="ot", name="ot")
                if last:
                    # chunk-major: each chunk's accumulation closes early so
                    # its add+DMA overlap the next chunk's matmuls; only the
                    # final 128-wide chunk is on the dependent tail
                    for k, ((off, w), pt) in enumerate(zip(cols, pts)):
                        for dt in range(DT):
                            nc.tensor.matmul(
                                pt[:],
                                weff_sb[:, dt, et * 128:(et + 1) * 128],
                                xT_sb[:, dt, off:off + w],
                                start=(dt == 0), stop=(dt == DT - 1),
                            )
                        if k == 1:
                            nc.vector.tensor_scalar_add(
                                ot[:, off:off + w], pt[:],
                                bias_sb[:, et:et + 1])
                        else:
                            nc.scalar.add(ot[:, off:off + w], pt[:],
                                          bias_sb[:, et:et + 1])
                        eng = nc.scalar if k == 2 else nc.sync
                        eng.dma_start(outT_r[:, et, off:off + w],
                                      ot[:, off:off + w])
                else:
                    for dt in range(DT):
                        lhs = weff_sb[:, dt, et * 128:(et + 1) * 128]
                        for (off, w), pt in zip(cols, pts):
                            nc.tensor.matmul(
                                pt[:], lhs, xT_sb[:, dt, off:off + w],
                                start=(dt == 0), stop=(dt == DT - 1),
                            )
                    for k, ((off, w), pt) in enumerate(zip(cols, pts)):
                        nc.scalar.add(ot[:, off:off + w], pt[:],
                                      bias_sb[:, et:et + 1])
                    nc.sync.dma_start(outT_r[:, et, :], ot[:])

    nc.compile()
    return nc


_NC_CACHE = None
LAST_EXEC_NS = None
LAST_RES = None


def kernel(x, w_qkv, w_fc, b_fc, _trace=False):
    global _NC_CACHE, LAST_EXEC_NS, LAST_RES
    x = np.asarray(x, dtype=np.float32)
    w_qkv = np.asarray(w_qkv, dtype=np.float32)
    w_fc = np.asarray(w_fc, dtype=np.float32)
    b_fc = np.asarray(b_fc, dtype=np.float32)

    if _NC_CACHE is None:
        _NC_CACHE = _build_program()
    nc = _NC_CACHE

    wq = np.ascontiguousarray(SCALE * w_qkv[:D]).astype(NPBF16)
    wkT = np.ascontiguousarray(w_qkv[D:2 * D].T).astype(NPBF16)
    wvT = np.ascontiguousarray(w_qkv[2 * D:].T).astype(NPBF16)
    wfcT = np.ascontiguousarray(w_fc.T).astype(NPBF16)

    in_maps = []
    for b in range(B):
        in_maps.append({
            "xN": x[b].astype(NPBF16),
            "xT": np.ascontiguousarray(x[b].T).astype(NPBF16),
            "wq": wq, "wkT": wkT, "wvT": wvT, "wfcT": wfcT,
            "bfc": b_fc,
        })

    res = bass_utils.run_bass_kernel_spmd(
        nc, in_maps, core_ids=list(range(B)), trace=_trace
    )
    LAST_EXEC_NS = res.exec_time_ns
    LAST_RES = res
    out = np.stack([res.results[b]["outT"].T.astype(np.float32)
                    for b in range(B)])
    return np.ascontiguousarray(out)
